# revision 1
# baseline (speedup 1.0000x reference)
"""Trainium2 Bass kernel for nn_LogLinearAttention (B=2,T=1024,Dm=1024,H=16,D=64,L=12).

Math (numpy-validated):
  out = ((S*Mw)@V / rowsum(S*Mw)) @ ow + ob   with S = phi(xQ) phi(xK)^T,
  Mw[i,j] = w~[i, lev(i,j)],  lev(i,j) = msb((i+1) XOR j)  (0-based, j<=i),
  w~ = exp(logits) (softmax cancels in num/den), phi(a) = max(a+1, min(exp(a),1)).

Cost-model-driven design (TimelineSim: serialized DMA engine, in-order
per-engine queues, fp16 matmuls 1 cycle/row vs f32r 4x below 256 cols):
 * fp16 end-to-end in DRAM/SBUF (f32 PSUM) halves the DMA stream and
   speeds every small matmul 4x; final absmax-rel ~3e-4 (tol 2e-2).
 * single x|wall DMA per 128-row chunk; o2 (Q,K,Q0,Q1) runs chunk-major so
   the PE streams behind the serialized input DMA; Q2/Q3 run slice-major
   afterward to fill the phi drain window.
 * K/w~ token-orientation via PE transposes packed into bitcast wide psum
   tiles; per-block REPLT constants carry the slot-127 level (no runtime
   WROW patches); row-127 inter-scale fixups via copy_predicated on wte.
 * one 8-bank [128,1024]xf32 "wide" psum tag serves every matmul target;
   inter contributions are copied to SBUF fp16 (ppsb) so the scaled
   accumulation (scalar_tensor_tensor per Fenwick segment) is cheap and
   the psum bank frees immediately.
 * the two batches are software-pipelined (batch-1 matmuls cover batch-0
   DVE windows); elementwise work is spread across DVE/ACT/Pool
   (gpsimd cannot touch PSUM; engines accept max one PSUM operand).

Sharding: 8 cores, core c owns heads {2c, 2c+1} for both batches
(tensor-parallel projections, head-parallel attention); per-core fp16
partial output projections are summed on host.
"""

from contextlib import ExitStack

import numpy as np

import concourse.bass as bass
import concourse.tile as tile
import concourse.mybir as mybir
from concourse import bacc
from concourse.bass_utils import run_bass_kernel_spmd

F32 = mybir.dt.float32
F16 = mybir.dt.float16
U8 = mybir.dt.uint8

B, T, DM, H, D, L = 2, 1024, 1024, 16, 64, 12
C = 128            # token block
NB = T // C        # 8
NCORES = 8
NTB = B * T // C   # 16 token blocks over (b, t)
KC = DM // 128     # 8 contraction chunks
NW = 412           # packed weights per chunk: qw 128 | kw 128 | V 132 | lw 24

AF = mybir.ActivationFunctionType
ALU = mybir.AluOpType


def _msb(v):
    return v.bit_length() - 1


def _decomp(bi):
    """Fenwick decomposition of block-prefix [0, bi): [(beta, size, g), ...]."""
    segs, start = [], 0
    for g in range(7, -1, -1):
        if (bi >> g) & 1:
            segs.append((start, 1 << g, g))
            start += 1 << g
    return segs


# state-tile layout: leaves P0..P6 at slots 0..6; combined segments:
_COMB = {(0, 2): 7, (0, 4): 8, (4, 2): 9}


def _l127(bi):
    return 7 + _msb((bi + 1) ^ bi)


def _build_slot_consts():
    """colind/rowind [128,128] fp16 and per-block replts [44, NB*128] fp16."""
    colind = np.zeros((128, C), np.float16)
    rowind = np.zeros((128, C), np.float16)
    replt = np.zeros((L, 128), np.float16)
    i1 = np.arange(1, C + 1)
    slot = 0
    for c in range(7):
        for m in range(1 << (6 - c)):
            rows = (((i1 >> (c + 1)) == m) & (((i1 >> c) & 1) == 1) & (i1 < C))
            rowind[slot, :] = rows.astype(np.float16)
            colind[slot, m * (1 << (c + 1)): m * (1 << (c + 1)) + (1 << c)] = 1.0
            replt[c, slot] = 1.0
            slot += 1
    assert slot == 127
    rowind[127, 127] = 1.0
    colind[127, :] = 1.0
    repl2 = np.zeros((44, NB * 128), np.float16)
    for bi in range(NB):
        rv = replt.copy()
        rv[_l127(bi), 127] = 1.0
        repl2[0:L, 128 * bi:128 * (bi + 1)] = rv
        repl2[32:32 + L, 128 * bi:128 * (bi + 1)] = rv
    return colind, rowind, repl2


def _w_fixups():
    """Row-127 level remaps on w~ for inter scale columns: [(bi, tgt, src)]."""
    fixes = []
    for bi in range(NB):
        for (beta, size, g) in _decomp(bi):
            tgt, src = 7 + g, 7 + _msb((bi + 1) ^ beta)
            if src != tgt:
                fixes.append((bi, tgt, src))
    return fixes


_PROGRAM_CACHE = {}


def _build_program(with_o1_bias: bool):
    nc = bacc.Bacc(trn_type="TRN2", target_bir_lowering=False, debug=False,
                   num_devices=NCORES)

    xw = nc.dram_tensor("xw", [DM, B * T + NW], F16,
                        kind="ExternalInput").ap()
    cvm = nc.dram_tensor("cvm", [128, 384 + 1024], F16,
                         kind="ExternalInput").ap()
    replts_d = nc.dram_tensor("replts", [44, NB * 128], F16,
                              kind="ExternalInput").ap()
    cvf = nc.dram_tensor("cvf", [128, 4], F32, kind="ExternalInput").ap()
    m127 = nc.dram_tensor("m127", [128, 1], U8, kind="ExternalInput").ap()
    bias1 = nc.dram_tensor("bias1", [128, 156], F32, kind="ExternalInput").ap()
    out_d = nc.dram_tensor("out", [B * T, DM], F16, kind="ExternalOutput").ap()

    fixes = _w_fixups()

    with tile.TileContext(nc) as tc, ExitStack() as ctx:
        const = ctx.enter_context(tc.tile_pool(name="const", bufs=1))
        big = ctx.enter_context(tc.tile_pool(name="big", bufs=1))
        sm = ctx.enter_context(tc.tile_pool(name="sm", bufs=3))
        acc = ctx.enter_context(tc.tile_pool(name="acc", bufs=2))

        # ---------- input DMAs: one merged x|wall transfer per chunk ------
        xwch = big.tile([128, KC, B * T + NW], F16)
        cvm_sb = const.tile([128, 384 + 1024], F16)
        replts_sb = const.tile([44, NB * 128], F16)
        cvf_sb = const.tile([128, 4], F32)
        m127_sb = const.tile([128, 1], U8)
        if with_o1_bias:
            bias1_sb = const.tile([128, 156], F32)
        for k in range(KC):
            if k == 0:
                # split first chunk so the first o2 matmul starts early
                nc.sync.dma_start(out=xwch[:, 0, 0:NW + 512],
                                  in_=xw[0:128, 0:NW + 512])
                nc.sync.dma_start(out=xwch[:, 0, NW + 512:NW + B * T],
                                  in_=xw[0:128, NW + 512:NW + B * T])
                nc.sync.dma_start(out=cvf_sb, in_=cvf)
            else:
                nc.sync.dma_start(out=xwch[:, k, :],
                                  in_=xw[128 * k:128 * (k + 1), :])
            if k == 2 and with_o1_bias:
                nc.sync.dma_start(out=bias1_sb, in_=bias1)
        nc.sync.dma_start(out=m127_sb, in_=m127)
        nc.sync.dma_start(out=cvm_sb, in_=cvm)
        nc.sync.dma_start(out=replts_sb, in_=replts_d)
        wall_sb = xwch[:, :, 0:NW]
        xch = xwch[:, :, NW:NW + B * T]
        colind = cvm_sb[:, 0:128]
        ident = cvm_sb[:, 256:384]
        ow_sb = cvm_sb[:, 384:1408]

        QpT = big.tile([128, B * T], F16)
        KpT = big.tile([128, B * T], F16)
        Kp1 = big.tile([128, NTB, 128], F16)
        VW1 = big.tile([128, NTB, 156], F16)
        wte = big.tile([128, NTB, 24], F16)
        wtT = big.tile([44, B * T], F16)
        attn_a = big.tile([128, NTB, 128], F16)

        # ======== unified pipeline: projections + attention ========
        with tc.tile_pool(name="psA", bufs=1, space="PSUM") as psA:
            def wide(nm):
                return psA.tile([128, 1024], F32, tag="wide", bufs=4, name=nm)

            # ---- o2 chunk-major: Q/K slices packed 2-per-wide ----
            # wide order K01,K23,Q01 chunk-major; Q23 slice-major afterward
            # (its matmuls fill the PE-idle window while phi drains)
            o2w = [wide(f"o2_{i}") for i in range(4)]
            WIDX = {0: 2, 1: 3, 2: 0, 3: 1}
            for k in range(KC):
                for s in [4, 5, 6, 7, 0, 1]:
                    d, sl = s // 4, s % 4
                    pt = o2w[WIDX[s // 2]][:, 512 * (s % 2):512 * (s % 2) + 512]
                    nc.tensor.matmul(
                        pt, wall_sb[:, k, 128 * d:128 * (d + 1)],
                        xch[:, k, 512 * sl:512 * (sl + 1)],
                        start=(k == 0), stop=(k == KC - 1),
                        skip_group_check=True)
            for s in [2, 3]:
                d, sl = s // 4, s % 4
                pt = o2w[WIDX[s // 2]][:, 512 * (s % 2):512 * (s % 2) + 512]
                for k in range(KC):
                    nc.tensor.matmul(
                        pt, wall_sb[:, k, 128 * d:128 * (d + 1)],
                        xch[:, k, 512 * sl:512 * (sl + 1)],
                        start=(k == 0), stop=(k == KC - 1),
                        skip_group_check=True)
            # phi (K slices first so Kp1 transposes can start early)
            for s in [4, 5, 6, 7, 0, 1, 2, 3]:
                d, sl = s // 4, s % 4
                pt = o2w[WIDX[s // 2]][:, 512 * (s % 2):512 * (s % 2) + 512]
                dst = QpT if d == 0 else KpT
                bcol, b1col = (0, 1) if d == 0 else (2, 3)
                et = sm.tile([128, 512], F16, tag="phi_et", bufs=3,
                             name=f"et{s}")
                nc.scalar.activation(et, pt, AF.Exp,
                                     bias=cvf_sb[:, bcol:bcol + 1])
                if d == 1 or sl >= 1:   # DVE-heavy (ACT is the bottleneck)
                    ec = sm.tile([128, 512], F16, tag="phi_ec", bufs=3,
                                 name=f"ec{s}")
                    nc.vector.tensor_scalar(out=ec, in0=et, scalar1=0.0,
                                            scalar2=1.0, op0=ALU.add,
                                            op1=ALU.min)
                    nc.vector.scalar_tensor_tensor(
                        out=dst[:, 512 * sl:512 * (sl + 1)], in0=pt,
                        scalar=cvf_sb[:, b1col:b1col + 1], in1=ec,
                        op0=ALU.add, op1=ALU.max)
                else:        # Q: ACT-heavy
                    a1 = sm.tile([128, 512], F16, tag="phi_ec", bufs=3,
                                 name=f"a1{s}")
                    nc.scalar.activation(a1, pt, AF.Identity,
                                         bias=cvf_sb[:, b1col:b1col + 1])
                    nc.vector.scalar_tensor_tensor(
                        out=dst[:, 512 * sl:512 * (sl + 1)], in0=et,
                        scalar=1.0, in1=a1, op0=ALU.min, op1=ALU.max)

            Kp1f = Kp1.rearrange("p b c -> p (b c)")

            def kp1_tr(b):
                trk = wide(f"trk{b}").bitcast(F16)
                for j in range(NB):
                    blk = b * NB + j
                    nc.tensor.transpose(trk[:, 128 * j:128 * (j + 1)],
                                        KpT[:, 128 * blk:128 * (blk + 1)],
                                        ident)
                nc.vector.tensor_copy(Kp1f[:, 1024 * b:1024 * (b + 1)],
                                      trk[:, 0:1024])

            def o1_half(b):
                for i in range(4):
                    w = wide(f"o1_{b}_{i}")
                    for j in range(2):
                        tb = b * NB + 2 * i + j
                        pt = w[:, 512 * j:512 * j + 156]
                        for k in range(KC):
                            nc.tensor.matmul(
                                pt, xch[:, k, 128 * tb:128 * (tb + 1)],
                                wall_sb[:, k, 256:412],
                                start=(k == 0), stop=(k == KC - 1),
                                skip_group_check=True)
                        if with_o1_bias:
                            nc.vector.tensor_add(pt, pt, bias1_sb)
                        if tb % 2 == 0:
                            nc.vector.tensor_copy(VW1[:, tb, :], pt)
                        else:
                            nc.scalar.copy(VW1[:, tb, :], pt)
                ones_ap = bass.AP(
                    tensor=VW1.tensor,
                    offset=VW1.offset + (b * NB) * 156 + 64,
                    ap=[[VW1.ap[0][0], 128], [156, NB], [66, 2]])
                nc.vector.memset(ones_ap, 1.0)
                nc.scalar.activation(wte[:, 8 * b:8 * (b + 1), :],
                                     VW1[:, 8 * b:8 * (b + 1), 132:156],
                                     AF.Exp)

            def wtt_half(b):
                trw = wide(f"trw{b}").bitcast(F16)
                for j in range(NB):
                    tb = b * NB + j
                    nc.tensor.transpose(trw[0:12, 128 * j:128 * (j + 1)],
                                        wte[:, tb, 0:12], ident)
                    nc.tensor.transpose(trw[32:44, 128 * j:128 * (j + 1)],
                                        wte[:, tb, 12:24], ident)
                nc.scalar.copy(wtT[0:12, 1024 * b:1024 * (b + 1)],
                               trw[0:12, 0:1024])
                nc.scalar.copy(wtT[32:44, 1024 * b:1024 * (b + 1)],
                               trw[32:44, 0:1024])

            def fixups_all():
                fs = NTB * 24
                for (bi, tgt, srcl) in fixes:
                    def _wcols(col):
                        return bass.AP(
                            tensor=wte.tensor,
                            offset=wte.offset + bi * 24 + col,
                            ap=[[fs, 128], [NB * 24, 2], [12, 2]])
                    mk = bass.AP(tensor=m127_sb.tensor, offset=m127_sb.offset,
                                 ap=[[1, 128], [0, 2], [0, 2]])
                    nc.vector.copy_predicated(out=_wcols(tgt), mask=mk,
                                              data=_wcols(srcl))

            STs, smdts, nums, ppws, ppsegs = {}, {}, {}, {}, {}

            def states(b):
                tb0 = b * NB
                ST = acc.tile([128, 10, 132], F16, tag="ST", bufs=2,
                              name=f"ST{b}")
                STf = ST.rearrange("p s c -> p (s c)")
                stA = wide(f"stA{b}")
                for j in range(6):
                    off = 132 * j if j < 3 else 512 + 132 * (j - 3)
                    nc.tensor.matmul(stA[:, off:off + 132],
                                     Kp1[:, tb0 + j, :],
                                     VW1[:, tb0 + j, 0:132],
                                     start=True, stop=True,
                                     skip_group_check=True)
                stB = wide(f"stB{b}")
                nc.tensor.matmul(stB[:, 0:132], Kp1[:, tb0 + 6, :],
                                 VW1[:, tb0 + 6, 0:132],
                                 start=True, stop=True, skip_group_check=True)
                nc.scalar.copy(STf[:, 0:396], stA[:, 0:396])
                nc.scalar.copy(STf[:, 396:792], stA[:, 512:908])
                nc.scalar.copy(STf[:, 792:924], stB[:, 0:132])
                nc.gpsimd.tensor_add(ST[:, 7, :], ST[:, 0, :], ST[:, 1, :])
                nc.gpsimd.tensor_add(ST[:, 8, :], ST[:, 7, :], ST[:, 2, :])
                nc.gpsimd.tensor_add(ST[:, 8, :], ST[:, 8, :], ST[:, 3, :])
                nc.gpsimd.tensor_add(ST[:, 9, :], ST[:, 4, :], ST[:, 5, :])
                STs[b] = ST

            def masks_scores(b):
                tb0 = b * NB
                wrs, sdts_ = {}, {}
                for h in range(2):
                    wr = wide(f"wr{b}{h}")
                    for bi in range(NB):
                        nc.tensor.matmul(
                            wr[:, 128 * bi:128 * (bi + 1)],
                            replts_sb[32 * h:32 * h + 12,
                                      128 * bi:128 * (bi + 1)],
                            wtT[32 * h:32 * h + 12,
                                1024 * b + 128 * bi:1024 * b + 128 * (bi + 1)],
                            start=True, stop=True, skip_group_check=True)
                    wrs[h] = wr
                for h in range(2):
                    hp = slice(64 * h, 64 * (h + 1))
                    sdt = wide(f"sdt{b}{h}")
                    for bi in range(NB):
                        tok = slice(C * (tb0 + bi), C * (tb0 + bi + 1))
                        nc.tensor.matmul(sdt[:, 128 * bi:128 * (bi + 1)],
                                         KpT[hp, tok], QpT[hp, tok],
                                         start=True, stop=True,
                                         skip_group_check=True)
                    sdts_[h] = sdt
                wrows = {}
                for h in range(2):
                    wrow = sm.tile([128, 8, 128], F16, tag="wrow", bufs=3,
                                   name=f"wrow{b}{h}")
                    rb = bass.AP(tensor=cvm_sb.tensor,
                                 offset=cvm_sb.offset + 128,
                                 ap=[[cvm_sb.ap[0][0], 128], [0, 8],
                                     [1, 128]])
                    nc.vector.tensor_tensor(
                        out=wrow, in0=wrs[h].rearrange("p (b c) -> p b c",
                                                       b=8),
                        in1=rb, op=ALU.mult)
                    wrows[h] = wrow
                return wrs, sdts_, wrows

            def mw_stage(b, wrows, sdts_):
                for h in (1, 0):
                    mw = wide(f"mw{b}{h}")
                    wrow_f = wrows[h].rearrange("p b c -> p (b c)")
                    for hf in range(2):
                        nc.tensor.matmul(
                            mw[:, 512 * hf:512 * (hf + 1)], colind,
                            wrow_f[:, 512 * hf:512 * (hf + 1)],
                            start=True, stop=True, skip_group_check=True)
                    mwsb = sm.tile([128, 8, 128], F16, tag="mwsb", bufs=3,
                                   name=f"mwsb{b}{h}")
                    nc.scalar.copy(mwsb.rearrange("p b c -> p (b c)"), mw)
                    smdt = sm.tile([128, 8, 128], F16, tag="smdt", bufs=3,
                                   name=f"smdt{b}{h}")
                    nc.vector.tensor_tensor(
                        out=smdt,
                        in0=sdts_[h].rearrange("p (b c) -> p b c", b=8),
                        in1=mwsb, op=ALU.mult)
                    smdts[(b, h)] = smdt

            def numint(b):
                tb0 = b * NB
                for h in (1, 0):
                    hp = slice(64 * h, 64 * (h + 1))
                    vc = slice(66 * h, 66 * (h + 1))
                    smdt = smdts[(b, h)]
                    num = acc.tile([128, 8, 66], F32, tag="num", bufs=4,
                                   name=f"num{b}{h}")
                    numf = num.rearrange("p b c -> p (b c)")
                    ndp = wide(f"nd{b}{h}")
                    for bi in range(NB):
                        blk = tb0 + bi
                        off = 66 * bi if bi < 4 else 512 + 66 * (bi - 4)
                        nc.tensor.matmul(ndp[:, off:off + 66],
                                         smdt[:, bi, :], VW1[:, blk, vc],
                                         start=True, stop=True,
                                         skip_group_check=True)
                    ppw = wide(f"pp{b}{h}")
                    segs = []
                    seg_i = 0
                    for bi in range(NB):
                        blk = tb0 + bi
                        tok = slice(C * blk, C * (blk + 1))
                        for (beta, sz, g) in _decomp(bi):
                            off = (66 * seg_i if seg_i < 6
                                   else 512 + 66 * (seg_i - 6))
                            si = beta if sz == 1 else _COMB[(beta, sz)]
                            nc.tensor.matmul(ppw[:, off:off + 66],
                                             QpT[hp, tok],
                                             STs[b][hp, si, vc],
                                             start=True, stop=True,
                                             skip_group_check=True)
                            segs.append((bi, blk, g, off))
                            seg_i += 1
                    nc.scalar.copy(numf[:, 0:264], ndp[:, 0:264])
                    nc.scalar.copy(numf[:, 264:528], ndp[:, 512:776])
                    ppsb = sm.tile([128, 1024], F16, tag="ppsb", bufs=6,
                                   name=f"ppsb{b}{h}")
                    if h == 1:
                        nc.vector.tensor_copy(ppsb[:, 0:396], ppw[:, 0:396])
                        nc.scalar.copy(ppsb[:, 512:908], ppw[:, 512:908])
                    else:
                        nc.scalar.copy(ppsb[:, 0:396], ppw[:, 0:396])
                        nc.vector.tensor_copy(ppsb[:, 512:908],
                                              ppw[:, 512:908])
                    nums[(b, h)] = num
                    ppws[(b, h)] = ppsb
                    ppsegs[(b, h)] = segs

            def stts(b):
                tb0 = b * NB
                for h in (1, 0):
                    num = nums[(b, h)]
                    ppw = ppws[(b, h)]
                    for (bi, blk, g, off) in ppsegs[(b, h)]:
                        sc = wte[:, blk, 12 * h + 7 + g:12 * h + 8 + g]
                        nc.vector.scalar_tensor_tensor(
                            out=num[:, bi, :], in0=ppw[:, off:off + 66],
                            scalar=sc, in1=num[:, bi, :],
                            op0=ALU.mult, op1=ALU.add)
                    dcol = sm.tile([128, 8], F32, tag="dcol", bufs=4,
                                   name=f"dcol{b}{h}")
                    nc.vector.tensor_copy(dcol, num[:, :, 64])
                    rec = sm.tile([128, 8], F32, tag="rec", bufs=4,
                                  name=f"rec{b}{h}")
                    nc.vector.reciprocal(rec, dcol)
                    rb2 = bass.AP(tensor=rec.tensor, offset=rec.offset,
                                  ap=[[rec.ap[0][0], 128], [1, 8], [0, 64]])
                    att = bass.AP(tensor=attn_a.tensor,
                                  offset=attn_a.offset + tb0 * 128 + 64 * h,
                                  ap=[[attn_a.ap[0][0], 128], [128, 8],
                                      [1, 64]])
                    nc.gpsimd.tensor_mul(att, num[:, :, 0:64], rb2)

            attnTs = {}

            def oproj_a(b):
                tb0 = b * NB
                tra = wide(f"tra{b}").bitcast(F16)
                for j in range(NB):
                    nc.tensor.transpose(tra[:, 128 * j:128 * (j + 1)],
                                        attn_a[:, tb0 + j, :], ident)
                attnT = sm.tile([128, 8, 128], F16, tag="attnT", bufs=3,
                                name=f"attnT{b}")
                nc.vector.tensor_copy(
                    attnT.rearrange("p b c -> p (b c)"), tra[:, 0:1024])
                attnTs[b] = attnT

            def oproj_b(b, act_only):
                tb0 = b * NB
                attnT = attnTs[b]
                for j in range(NB):
                    blk = tb0 + j
                    po = wide(f"po{b}{j}")
                    for half in range(2):
                        nc.tensor.matmul(
                            po[:, 512 * half:512 * (half + 1)],
                            attnT[:, j, :],
                            ow_sb[:, 512 * half:512 * (half + 1)],
                            start=True, stop=True, skip_group_check=True)
                    ot = sm.tile([128, 1024], F16, tag="ot", bufs=8,
                                 name=f"ot{b}{j}")
                    dve0 = (j % 2 == 1) if not act_only else (j % 4 == 1)
                    dve1 = (j % 2 == 0) if not act_only else (j % 4 == 3)
                    if dve0:
                        nc.vector.tensor_copy(ot[:, 0:512], po[:, 0:512])
                    else:
                        nc.scalar.copy(ot[:, 0:512], po[:, 0:512])
                    if dve1:
                        nc.vector.tensor_copy(ot[:, 512:1024],
                                              po[:, 512:1024])
                    else:
                        nc.scalar.copy(ot[:, 512:1024], po[:, 512:1024])
                    nc.sync.dma_start(out=out_d[C * blk:C * (blk + 1), :],
                                      in_=ot)

            # ---- schedule ----
            kp1_tr(0)
            o1_half(0)
            states(0)
            wtt_half(0)
            o1_half(1)
            wrs0, sdts0, wrows0 = masks_scores(0)
            mw_stage(0, wrows0, sdts0)
            numint(0)
            kp1_tr(1)
            wtt_half(1)
            fixups_all()
            states(1)
            wrs1, sdts1, wrows1 = masks_scores(1)
            stts(0)
            mw_stage(1, wrows1, sdts1)
            numint(1)
            oproj_a(0)
            stts(1)
            oproj_b(0, act_only=True)
            oproj_a(1)
            oproj_b(1, act_only=False)

    nc.compile()
    return nc


def _host_prep(inputs):
    x = np.asarray(inputs["x"], np.float32).reshape(B * T, DM)
    xT16 = np.ascontiguousarray(x.T.astype(np.float16))
    qw = np.asarray(inputs["qw"], np.float32)
    kw = np.asarray(inputs["kw"], np.float32)
    vw = np.asarray(inputs["vw"], np.float32)
    lw = np.asarray(inputs["lw"], np.float32)
    ow = np.asarray(inputs["ow"], np.float32)
    qb = np.asarray(inputs["qb"], np.float32)
    kb = np.asarray(inputs["kb"], np.float32)
    vb = np.asarray(inputs["vb"], np.float32)
    lb = np.asarray(inputs["lb"], np.float32)

    colind, rowind, replts = _build_slot_consts()
    m127_host = np.zeros((128, 1), np.uint8)
    m127_host[127, 0] = 1

    in_maps = []
    for c in range(NCORES):
        hA, hB = 2 * c, 2 * c + 1
        wallh = np.zeros((DM, NW), np.float16)
        wallh[:, 0:128] = qw[:, 128 * c:128 * (c + 1)].astype(np.float16)
        wallh[:, 128:256] = kw[:, 128 * c:128 * (c + 1)].astype(np.float16)
        wallh[:, 256:320] = vw[:, 128 * c:128 * c + 64].astype(np.float16)
        wallh[:, 322:386] = vw[:, 128 * c + 64:128 * (c + 1)].astype(np.float16)
        wallh[:, 388:400] = lw[:, 12 * hA:12 * hA + 12].astype(np.float16)
        wallh[:, 400:412] = lw[:, 12 * hB:12 * hB + 12].astype(np.float16)
        cvmh = np.zeros((128, 384), np.float16)
        cvmh[:, 0:128] = colind
        cvmh[:, 128:256] = rowind
        cvmh[:, 256:384] = np.eye(128, dtype=np.float16)
        owh = ow[128 * c:128 * (c + 1), :].astype(np.float16)
        cvfh = np.zeros((128, 4), np.float32)
        cvfh[:, 0] = qb[128 * c:128 * (c + 1)]
        cvfh[:, 1] = qb[128 * c:128 * (c + 1)] + 1.0
        cvfh[:, 2] = kb[128 * c:128 * (c + 1)]
        cvfh[:, 3] = kb[128 * c:128 * (c + 1)] + 1.0
        bias1h = np.zeros((128, 156), np.float32)
        bias1h[:, 0:64] = vb[128 * c:128 * c + 64]
        bias1h[:, 66:130] = vb[128 * c + 64:128 * (c + 1)]
        bias1h[:, 132:144] = lb[12 * hA:12 * hA + 12]
        bias1h[:, 144:156] = lb[12 * hB:12 * hB + 12]
        xwh = np.concatenate([wallh, xT16], axis=1)
        cvm2 = np.concatenate([cvmh, owh], axis=1)
        in_maps.append({
            "xw": np.ascontiguousarray(xwh),
            "cvm": np.ascontiguousarray(cvm2),
            "replts": np.ascontiguousarray(replts),
            "cvf": cvfh,
            "m127": m127_host,
            "bias1": bias1h,
        })
    with_bias = bool(np.any(vb) or np.any(kb) or np.any(lb))
    return in_maps, with_bias


def kernel(**inputs) -> np.ndarray:
    in_maps, with_bias = _host_prep(inputs)
    if with_bias not in _PROGRAM_CACHE:
        _PROGRAM_CACHE[with_bias] = _build_program(with_bias)
    nc = _PROGRAM_CACHE[with_bias]
    res = run_bass_kernel_spmd(nc, in_maps, list(range(NCORES)))
    ob = np.asarray(inputs["ob"], np.float32)
    out = np.zeros((B * T, DM), np.float32)
    for r in res.results:
        out += np.asarray(r["out"], np.float32)
    out += ob[None, :]
    return out.reshape(B, T, DM)

